# revision 1
# baseline (speedup 1.0000x reference)
"""Bass/Tile TRN2 kernel for nn_MultiHead (B=4, T=2048, C=1024, H=16, D=64).

Sharding: 8 cores = batch(4) x head-group(2).  Each core computes, for its
batch b and its 8 heads, the full attention block and a *partial* output
projection (its 512 rows of Wo).  Host sums the two partials per batch and
adds the bias.

On-device layout trick: all activations are kept transposed
([feature, time]) so every matmul sees natural-layout operands:
  - q/k/v are DMA-transposed on load (bf16, xbar transpose), in 512-col chunks
  - q-proj / k-proj:  lhsT = Wq[cb]  (c,hd),  rhs = qT[cb] (c,t)  -> qh^T (hd,t)
  - v-proj:           lhsT = vT[cb]  (c,tk),  rhs = Wv[cb] (c,hd) -> vh (tk,hd)
  - QK^T:             lhsT = kh^T    (d,tk),  rhs = qh^T   (d,tq) -> logits^T (tk,tq)
    (two heads run concurrently via PE row-tiling: d=64 halves of the array;
     both land in one 2-bank PSUM tile so one ACT exp covers both heads)
  - softmax along partitions: ones-column appended to vh gives row sums for
    free inside the PV matmul; causal mask added before exp; exp on ACT
  - PV:               lhsT = vh_aug  (tk,65), rhs = P^T    (tk,tq) -> [pv^T; S] (65,tq)
  - normalize: inv = 1/S broadcast to 64 partitions via a K=1 matmul
  - out-proj:         lhsT = att^T   (hd,tq), rhs = Wo     (hd,c)  -> out (tq,c)
Attention runs tq-chunk-major with the out-projection interleaved between
head-pairs to fill PE bubbles in the ACT(exp)-bound inner loop.
"""

import numpy as np
import ml_dtypes

B, T, C, H, D = 4, 2048, 1024, 16, 64
NCORES = 8
HPC = H // 2            # heads per core
HD = HPC * D            # 512, hidden per core
NPAIR = HPC // 2        # 4 head pairs
NCB = C // 128          # 8 contraction blocks
NTQ = T // 512          # 4 tq chunks
NTKB = T // 128         # 16 tk blocks
SCALE = float(1.0 / np.sqrt(np.float32(C)))
PIPE = 3                # QK->PV software pipeline depth

BF16 = ml_dtypes.bfloat16

_CACHE = {}


def build_program(repeat=1, stages='all'):
    """Build + compile the per-core Bass program (cached)."""
    key = ("nc", repeat, stages)
    if key in _CACHE:
        return _CACHE[key]

    import concourse.mybir as mybir
    import concourse.tile as tile
    from concourse import bacc
    from contextlib import ExitStack

    BF = mybir.dt.bfloat16
    F32 = mybir.dt.float32

    nc = bacc.Bacc("TRN2", target_bir_lowering=False, debug=False,
                   enable_asserts=False, num_devices=NCORES)

    xq = nc.dram_tensor("xq", [T, C], BF, kind="ExternalInput").ap()
    xk = nc.dram_tensor("xk", [T, C], BF, kind="ExternalInput").ap()
    xv = nc.dram_tensor("xv", [T, C], BF, kind="ExternalInput").ap()
    wq = nc.dram_tensor("wq", [C, HD], BF, kind="ExternalInput").ap()
    wk = nc.dram_tensor("wk", [C, HD], BF, kind="ExternalInput").ap()
    wv = nc.dram_tensor("wv", [C, HD], BF, kind="ExternalInput").ap()
    wo = nc.dram_tensor("wo", [HD, C], BF, kind="ExternalInput").ap()
    out = nc.dram_tensor("out", [T, C], F32, kind="ExternalOutput").ap()

    EXP = mybir.ActivationFunctionType.Exp

    with tile.TileContext(nc) as tc:
        with ExitStack() as ctx:
            if repeat > 1:
                ctx.enter_context(tc.For_i(0, repeat, 1))
            res = ctx.enter_context(tc.tile_pool(name="res", bufs=1))
            xTp = ctx.enter_context(tc.tile_pool(name="xT", bufs=24))
            ptp = ctx.enter_context(tc.tile_pool(name="pt", bufs=4))
            sbp = ctx.enter_context(tc.tile_pool(name="sb", bufs=4))
            outp = ctx.enter_context(tc.tile_pool(name="outS", bufs=2))
            mmA = tc.alloc_tile_pool(name="mmA", bufs=2, space="PSUM")

            # ---- transposed input loads (kick q's first chunk ASAP) --------
            xT = {}   # (tensor, tchunk, cb) -> tile

            def load_xT(key, src, j):
                for cb in range(NCB):
                    t = xTp.tile([128, 512], BF, tag="xT", name="xT")
                    eng = nc.sync
                    eng.dma_start(
                        t[:], src[j * 512:(j + 1) * 512, cb * 128:(cb + 1) * 128],
                        transpose=True)
                    xT[(key, j, cb)] = t

            # ---- weights (wq first; others interleaved with loads) ----
            # each weight tensor loads as ONE strided DMA into one wide tile
            def load_w(dst, src_ap, nblk, width, eng):
                # dst[r, b*width+h] = src[b*128+r, h]
                eng.dma_start(
                    dst[:].rearrange("p (b h) -> p b h", h=width),
                    src_ap.rearrange("(b p) h -> p b h", p=128))

            wqall = res.tile([128, NCB * HD], BF, tag="wqall", name="wqall")
            load_w(wqall, wq, NCB, HD, nc.scalar)
            wq_sb = [wqall[:, cb * HD:(cb + 1) * HD] for cb in range(NCB)]
            for j in range(NTQ if not stages.startswith('attn') else 0):
                load_xT("q", xq, j)

            # ---- constants --------------------------------------------------
            maskT = res.tile([128, 128], BF, tag="maskT", name="maskT")
            nc.gpsimd.memset(maskT[:], 1.0)
            # maskT[i,j] = 0 where j < i (strictly-lower = future in [tk,tq])
            nc.gpsimd.affine_select(
                out=maskT[:], in_=maskT[:],
                compare_op=mybir.AluOpType.is_ge, fill=0.0,
                base=0, pattern=[[1, 128]], channel_multiplier=-1,
            )
            ones64 = res.tile([1, 64], F32, tag="ones64", name="ones64")
            nc.vector.memset(ones64[:], 1.0)

            # ---- resident activations --------------------------------------
            qhT = [res.tile([128, T], BF, tag=f"qhT{p}", name=f"qhT{p}")
                   for p in range(NPAIR)]
            khT = [res.tile([128, T], BF, tag=f"khT{p}", name=f"khT{p}")
                   for p in range(NPAIR)]
            vha = [res.tile([128, HPC * 65], BF, tag=f"vha{kb}", name=f"vha{kb}")
                   for kb in range(NTKB)]
            attT = [res.tile([128, T], BF, tag=f"attT{p}", name=f"attT{p}")
                    for p in range(NPAIR)]

            if stages.startswith('attn'):
                for p in range(NPAIR):
                    nc.vector.memset(qhT[p][:], 0.01)
                    nc.vector.memset(khT[p][:], 0.01)
                    nc.vector.memset(attT[p][:], 0.0)
                for kb in range(NTKB):
                    nc.vector.memset(vha[kb][:], 0.01)

            # ---- projections (tchunk-major frees xT tiles early) -----------
            def qk_proj(key, w_tiles, dstT):
                for j in range(NTQ):
                    for p in range(NPAIR):
                        ps = mmA.tile([128, 512], F32, tag="mmA", name="mmA")
                        for cb in range(NCB):
                            nc.tensor.matmul(
                                ps[:],
                                lhsT=w_tiles[cb][:, p * 128:(p + 1) * 128],
                                rhs=xT[(key, j, cb)][:],
                                start=(cb == 0), stop=(cb == NCB - 1))
                        nc.vector.tensor_copy(
                            dstT[p][:, j * 512:(j + 1) * 512], ps[:])

            if not stages.startswith('attn'):
                qk_proj("q", wq_sb, qhT)
            wkall = res.tile([128, NCB * HD], BF, tag="wkall", name="wkall")
            load_w(wkall, wk, NCB, HD, nc.scalar)
            wk_sb = [wkall[:, cb * HD:(cb + 1) * HD] for cb in range(NCB)]
            if not stages.startswith('attn'):
                for j in range(NTQ):
                    load_xT("k", xk, j)
                qk_proj("k", wk_sb, khT)

            wvall = res.tile([128, NCB * HD], BF, tag="wvall", name="wvall")
            load_w(wvall, wv, NCB, HD, nc.scalar)
            wv_sb = [wvall[:, cb * HD:(cb + 1) * HD] for cb in range(NCB)]
            woall = res.tile([128, (HD // 128) * C], BF, tag="woall",
                             name="woall")
            load_w(woall, wo, HD // 128, C, nc.scalar)
            wo_sb = [woall[:, hb * C:(hb + 1) * C] for hb in range(HD // 128)]
            # v-proj emitted as small thunks: chunks 0-1 upfront, 2-3 as
            # fillers inside the attention loop (keeps PE busy while exp
            # paces the inner loop, and spreads the v transposes out).
            def vproj_thunks(kb, pool=None, tag="mm"):
                st = {}

                def mk_mm(cb):
                    def f():
                        if cb == 0:
                            st["ps"] = pool.tile([128, 512], F32, tag=tag,
                                                 name=tag)
                        nc.tensor.matmul(
                            st["ps"][:],
                            lhsT=xT[("v", kb // 4, cb)][:, (kb % 4) * 128:
                                                        (kb % 4 + 1) * 128],
                            rhs=wv_sb[cb][:],
                            start=(cb == 0), stop=(cb == NCB - 1))
                    return f

                def evac():
                    for h in range(HPC):
                        nc.vector.tensor_copy(
                            vha[kb][:, h * 65:h * 65 + 64],
                            st["ps"][:, h * 64:(h + 1) * 64])
                        nc.vector.memset(
                            vha[kb][:, h * 65 + 64:h * 65 + 65], 1.0)

                return [mk_mm(cb) for cb in range(NCB)] + [evac]

            n_vchunk_upfront = NTQ if stages == 'proj' else 2
            if not stages.startswith('attn'):
                for j in range(n_vchunk_upfront):
                    load_xT("v", xv, j)
                for kb in range(4 * n_vchunk_upfront):
                    for f in vproj_thunks(kb, pool=mmA, tag="mmA"):
                        f()

            # ---- phase switch: release proj PSUM, open attention pools -----
            mmA.release()
            lgp = ctx.enter_context(tc.tile_pool(name="lg", bufs=2,
                                                 space="PSUM"))
            accp = ctx.enter_context(tc.tile_pool(name="acc", bufs=3,
                                                  space="PSUM"))
            mmp = ctx.enter_context(tc.tile_pool(name="mm", bufs=1,
                                                 space="PSUM"))

            # ---- attention + interleaved fillers ---------------------------
            filler = []

            def pop_fillers(n=2):
                for _ in range(min(n, len(filler))):
                    filler.pop(0)()

            def attention(p, j):
                c1 = (2 * p) * 65
                c2 = (2 * p + 1) * 65
                ntk = 4 * (j + 1)
                acc1 = accp.tile([65, 512], F32, tag="acc", name="acc")
                acc2 = accp.tile([65, 512], F32, tag="acc", name="acc")
                pend = []

                def emit_pv(it):
                    kb, o, w, pt = it
                    nc.tensor.matmul(
                        acc1[:, o:512], lhsT=vha[kb][:, c1:c1 + 65],
                        rhs=pt[:, 0:w],
                        start=(kb == 0), stop=(kb == ntk - 1))
                    nc.tensor.matmul(
                        acc2[:, o:512], lhsT=vha[kb][:, c2:c2 + 65],
                        rhs=pt[:, 512:512 + w],
                        start=(kb == 0), stop=(kb == ntk - 1))

                for kb in range(ntk):
                    o = max(0, kb * 128 - j * 512)
                    w = 512 - o
                    lg = lgp.tile([128, 1024], F32, tag="lg", name="lg")
                    nc.tensor.matmul(
                        lg[:, 0:w],
                        lhsT=khT[p][0:64, kb * 128:(kb + 1) * 128],
                        rhs=qhT[p][0:64, j * 512 + o:(j + 1) * 512],
                        start=True, stop=True)
                    nc.tensor.matmul(
                        lg[:, 512:512 + w],
                        lhsT=khT[p][64:128, kb * 128:(kb + 1) * 128],
                        rhs=qhT[p][64:128, j * 512 + o:(j + 1) * 512],
                        start=True, stop=True)
                    pt = ptp.tile([128, 1024], BF, tag="pt", name="pt")
                    if kb >= 4 * j:  # diagonal block: 2 narrow exps + 0/1 mask
                        nc.scalar.activation(
                            pt[:, 0:w], lg[:, 0:w], EXP, scale=SCALE)
                        nc.scalar.activation(
                            pt[:, 512:512 + w], lg[:, 512:512 + w], EXP,
                            scale=SCALE)
                        nc.vector.tensor_mul(
                            pt[:, 0:128], pt[:, 0:128], maskT[:])
                        nc.vector.tensor_mul(
                            pt[:, 512:640], pt[:, 512:640], maskT[:])
                    else:            # one exp covering both heads
                        nc.scalar.activation(
                            pt[:, 0:1024], lg[:, 0:1024], EXP, scale=SCALE)
                    pend.append((kb, o, w, pt))
                    pop_fillers(2)
                    if len(pend) > PIPE:
                        emit_pv(pend.pop(0))
                for it in pend:
                    emit_pv(it)

                # normalize: att^T = pv^T * (1/S), bf16
                if stages == 'attn_nonorm':
                    for acc in (acc1, acc2):
                        inv = sbp.tile([1, 512], F32, tag="inv", name="inv")
                        nc.vector.tensor_copy(inv[:], acc[64:65, :])
                elif stages == 'attn_nm2':
                    for acc, row in ((acc1, 0), (acc2, 64)):
                        pv = sbp.tile([65, 512], F32, tag="pvE", name="pvE")
                        nc.vector.tensor_copy(pv[:], acc[:])
                        lns = sbp.tile([1, 512], F32, tag="lns", name="lns")
                        nc.scalar.activation(
                            lns[:], pv[64:65, :],
                            mybir.ActivationFunctionType.Ln)
                        inv = sbp.tile([1, 512], F32, tag="inv", name="inv")
                        nc.scalar.activation(
                            inv[:], lns[:],
                            mybir.ActivationFunctionType.Exp, scale=-1.0)
                else:
                    for acc, row in ((acc1, 0), (acc2, 64)):
                        # 1/S as exp(-ln S) on ACT: DVE's iterative-divide
                        # reciprocal is ~8 cyc/elem on one lane and blocks
                        # the DVE FIFO for ~4us
                        pv = sbp.tile([65, 512], F32, tag="pvE", name="pvE")
                        nc.vector.tensor_copy(pv[:], acc[:])
                        lns = sbp.tile([1, 512], F32, tag="lns", name="lns")
                        nc.scalar.activation(
                            lns[:], pv[64:65, :],
                            mybir.ActivationFunctionType.Ln)
                        inv = sbp.tile([1, 512], F32, tag="inv", name="inv")
                        nc.scalar.activation(
                            inv[:], lns[:],
                            mybir.ActivationFunctionType.Exp, scale=-1.0)

                        # defer broadcast+scale: a bcast matmul emitted here
                        # would stall PE on the ACT ln/exp chain
                        def norm_thunk(pv=pv, inv=inv, row=row, p=p, j=j):
                            rep = mmp.tile([64, 512], F32, tag="mm",
                                           name="mm")
                            nc.tensor.matmul(rep[:], lhsT=ones64[:],
                                             rhs=inv[:],
                                             start=True, stop=True)
                            nc.vector.tensor_mul(
                                attT[p][row:row + 64,
                                        j * 512:(j + 1) * 512],
                                pv[0:64, :], rep[:])
                        filler.insert(0, norm_thunk)

            def outproj_thunks(tb):
                st = {}
                NHB = HD // 128

                def mk_mm(cc, hb):
                    def f():
                        if cc == 0 and hb == 0:
                            st["st"] = outp.tile([128, C], F32, tag="outS",
                                                 name="outS")
                        if hb == 0:
                            st["ps"] = mmp.tile([128, 512], F32, tag="mm",
                                                name="mm")
                        nc.tensor.matmul(
                            st["ps"][:],
                            lhsT=attT[hb][:, tb * 128:(tb + 1) * 128],
                            rhs=wo_sb[hb][:, cc * 512:(cc + 1) * 512],
                            start=(hb == 0), stop=(hb == NHB - 1))
                    return f

                def mk_evac(cc):
                    def f():
                        nc.vector.tensor_copy(
                            st["st"][:, cc * 512:(cc + 1) * 512], st["ps"][:])
                        if cc == C // 512 - 1:
                            nc.scalar.dma_start(
                                out[tb * 128:(tb + 1) * 128, :], st["st"][:])
                    return f

                th = []
                for cc in range(C // 512):
                    th.extend(mk_mm(cc, hb) for hb in range(NHB))
                    th.append(mk_evac(cc))
                return th

            if stages == 'proj':
                for p in range(NPAIR):
                    nc.vector.memset(attT[p][:], 0.0)
                for tb in range(T // 128):
                    for f in outproj_thunks(tb):
                        f()
            else:
                nofill = (stages == 'attn_nofill')
                for j in range(NTQ):
                    if not stages.startswith('attn') and j + 2 < NTQ:
                        load_xT("v", xv, j + 2)
                    if not stages.startswith('attn'):
                        for kb in range(4 * (j + 2), 4 * (j + 3)):
                            if kb < NTKB:
                                filler.extend(vproj_thunks(kb, pool=mmp, tag="mm"))
                    if j > 0 and not nofill:
                        for tb in range(4 * (j - 1), 4 * j):
                            filler.extend(outproj_thunks(tb))
                    for p in range(NPAIR):
                        attention(p, j)
                if nofill:
                    for tb in range(NTKB):
                        filler.extend(outproj_thunks(tb))
                else:
                    for tb in range(4 * (NTQ - 1), NTKB):
                        filler.extend(outproj_thunks(tb))
                while filler:
                    pop_fillers(8)

    nc.compile()
    _CACHE[key] = nc
    return nc


def make_in_maps(q, k, v, Wq, Wk, Wv, Wo):
    q = np.asarray(q, np.float32)
    k = np.asarray(k, np.float32)
    v = np.asarray(v, np.float32)
    Wq = np.asarray(Wq, np.float32)
    Wk = np.asarray(Wk, np.float32)
    Wv = np.asarray(Wv, np.float32)
    Wo = np.asarray(Wo, np.float32)

    def wslice(W, g):  # [H,C,D] -> [C, 8*D] for head group g
        return np.ascontiguousarray(
            W[g * HPC:(g + 1) * HPC].transpose(1, 0, 2).reshape(C, HD)
        ).astype(BF16)

    maps = []
    for core in range(NCORES):
        b, g = core // 2, core % 2
        maps.append({
            "xq": q[b].astype(BF16),
            "xk": k[b].astype(BF16),
            "xv": v[b].astype(BF16),
            "wq": wslice(Wq, g),
            "wk": wslice(Wk, g),
            "wv": wslice(Wv, g),
            "wo": np.ascontiguousarray(Wo[g * HD:(g + 1) * HD]).astype(BF16),
        })
    return maps


def kernel(q, k, v, Wq, Wk, Wv, Wo, bo):
    from concourse.bass_utils import run_bass_kernel_spmd

    nc = build_program()
    in_maps = make_in_maps(q, k, v, Wq, Wk, Wv, Wo)
    res = run_bass_kernel_spmd(nc, in_maps, list(range(NCORES))).results
    bo = np.asarray(bo, np.float32)
    outv = np.empty((B, T, C), np.float32)
    for b in range(B):
        outv[b] = res[2 * b]["out"] + res[2 * b + 1]["out"]
    outv += bo
    return outv



# revision 8
# speedup vs baseline: 2.8619x; 2.8619x over previous
"""Bass/Tile TRN2 kernel for nn_MultiHead (B=4, T=2048, C=1024, H=16, D=64).

Sharding: 8 cores = batch(4) x head-group(2).  Each core computes, for its
batch b and its 8 heads, the full attention block and a *partial* output
projection (its 512 rows of Wo).  Host sums the two partials per batch and
adds the bias.

On-device layout trick: all activations are kept transposed
([feature, time]) so every matmul sees natural-layout operands:
  - q/k/v are DMA-transposed on load (bf16, xbar transpose), in 512-col chunks
  - q-proj / k-proj:  lhsT = Wq[cb]  (c,hd),  rhs = qT[cb] (c,t)  -> qh^T (hd,t)
  - v-proj:           lhsT = vT[cb]  (c,tk),  rhs = Wv[cb] (c,hd) -> vh (tk,hd)
  - QK^T:             lhsT = kh^T    (d,tk),  rhs = qh^T   (d,tq) -> logits^T (tk,tq)
    (two heads run concurrently via PE row-tiling: d=64 halves of the array;
     both land in one 2-bank PSUM tile so one ACT exp covers both heads)
  - softmax along partitions: ones-column appended to vh gives row sums for
    free inside the PV matmul; causal mask added before exp; exp on ACT
  - PV:               lhsT = vh_aug  (tk,65), rhs = P^T    (tk,tq) -> [pv^T; S] (65,tq)
  - normalize: inv = 1/S broadcast to 64 partitions via a K=1 matmul
  - out-proj:         lhsT = att^T   (hd,tq), rhs = Wo     (hd,c)  -> out (tq,c)
Attention runs tq-chunk-major with the out-projection interleaved between
head-pairs to fill PE bubbles in the ACT(exp)-bound inner loop.
"""

import numpy as np
import ml_dtypes

B, T, C, H, D = 4, 2048, 1024, 16, 64
NCORES = 8
HPC = H // 2            # heads per core
HD = HPC * D            # 512, hidden per core
NPAIR = HPC // 2        # 4 head pairs
NCB = C // 128          # 8 contraction blocks
NTQ = T // 512          # 4 tq chunks
NTKB = T // 128         # 16 tk blocks
SCALE = float(1.0 / np.sqrt(np.float32(C)))
PIPE = 3                # QK->PV software pipeline depth

BF16 = ml_dtypes.bfloat16

_CACHE = {}


def build_program(repeat=1, stages='all'):
    """Build + compile the per-core Bass program (cached)."""
    key = ("nc", repeat, stages)
    if key in _CACHE:
        return _CACHE[key]

    import concourse.mybir as mybir
    import concourse.tile as tile
    from concourse import bacc
    from contextlib import ExitStack

    # Exp and Ln both live in the natural_log_exp_and_others table set, but
    # the table-load pass maps each func to the first set containing it
    # (Exp->exp_and_others, Ln->natural_log), reloading tables (~1.3us each)
    # at every Exp<->Ln switch.  Narrow the func->set map (dict order and
    # hence act_func_set_id indices unchanged) so both resolve to the shared
    # set and the load hoists out of the loop.
    from concourse.hw_specs import get_activation_tables
    _tabs = get_activation_tables("gen3")
    for _name, _funcs in _tabs.items():
        if _name != "natural_log_exp_and_others":
            _funcs.discard(mybir.ActivationFunctionType.Exp)
            _funcs.discard(mybir.ActivationFunctionType.Ln)

    BF = mybir.dt.bfloat16
    F32 = mybir.dt.float32

    nc = bacc.Bacc("TRN2", target_bir_lowering=False, debug=False,
                   enable_asserts=False, num_devices=NCORES)

    xq = nc.dram_tensor("xq", [T, C], BF, kind="ExternalInput").ap()
    xk = nc.dram_tensor("xk", [T, C], BF, kind="ExternalInput").ap()
    xv = nc.dram_tensor("xv", [T, C], BF, kind="ExternalInput").ap()
    wq = nc.dram_tensor("wq", [C, HD], BF, kind="ExternalInput").ap()
    wk = nc.dram_tensor("wk", [C, HD], BF, kind="ExternalInput").ap()
    wv = nc.dram_tensor("wv", [C, HD], BF, kind="ExternalInput").ap()
    wo = nc.dram_tensor("wo", [HD, C], BF, kind="ExternalInput").ap()
    out = nc.dram_tensor("out", [T, C], F32, kind="ExternalOutput").ap()

    EXP = mybir.ActivationFunctionType.Exp

    with tile.TileContext(nc) as tc:
        with ExitStack() as ctx:
            if repeat > 1:
                ctx.enter_context(tc.For_i(0, repeat, 1))
            res = ctx.enter_context(tc.tile_pool(name="res", bufs=1))
            xTp = ctx.enter_context(tc.tile_pool(name="xT", bufs=24))
            ptp = ctx.enter_context(tc.tile_pool(name="pt", bufs=4))
            sbp = ctx.enter_context(tc.tile_pool(name="sb", bufs=4))
            outp = ctx.enter_context(tc.tile_pool(name="outS", bufs=2))
            mmA = tc.alloc_tile_pool(name="mmA", bufs=2, space="PSUM")

            # ---- transposed input loads (kick q's first chunk ASAP) --------
            xT = {}   # (tensor, tchunk, cb) -> tile

            def load_xT(key, src, j):
                for cb in range(NCB):
                    t = xTp.tile([128, 512], BF, tag="xT", name="xT")
                    eng = nc.sync
                    eng.dma_start(
                        t[:], src[j * 512:(j + 1) * 512, cb * 128:(cb + 1) * 128],
                        transpose=True)
                    xT[(key, j, cb)] = t

            # ---- weights (wq first; others interleaved with loads) ----
            # each weight tensor loads as ONE strided DMA into one wide tile
            def load_w(dst, src_ap, nblk, width, eng):
                # dst[r, b*width+h] = src[b*128+r, h]
                eng.dma_start(
                    dst[:].rearrange("p (b h) -> p b h", h=width),
                    src_ap.rearrange("(b p) h -> p b h", p=128))

            wqall = res.tile([128, NCB * HD], BF, tag="wqall", name="wqall")
            load_w(wqall, wq, NCB, HD, nc.scalar)
            wq_sb = [wqall[:, cb * HD:(cb + 1) * HD] for cb in range(NCB)]
            for j in range(NTQ if not stages.startswith('attn') else 0):
                load_xT("q", xq, j)

            # ---- constants --------------------------------------------------
            maskT = res.tile([128, 128], BF, tag="maskT", name="maskT")
            nc.gpsimd.memset(maskT[:], 1.0)
            # maskT[i,j] = 0 where j < i (strictly-lower = future in [tk,tq])
            nc.gpsimd.affine_select(
                out=maskT[:], in_=maskT[:],
                compare_op=mybir.AluOpType.is_ge, fill=0.0,
                base=0, pattern=[[1, 128]], channel_multiplier=-1,
            )
            ones64 = res.tile([1, 64], BF, tag="ones64", name="ones64")
            nc.vector.memset(ones64[:], 1.0)

            # ---- resident activations --------------------------------------
            qhT = [res.tile([128, T], BF, tag=f"qhT{p}", name=f"qhT{p}")
                   for p in range(NPAIR)]
            khT = [res.tile([128, T], BF, tag=f"khT{p}", name=f"khT{p}")
                   for p in range(NPAIR)]
            vha = [res.tile([128, HPC * 65], BF, tag=f"vha{kb}", name=f"vha{kb}")
                   for kb in range(NTKB)]
            for kb in range(NTKB):
                # ones columns (col 64 of each head's 65-wide strip) are
                # written once here; the v-proj evac never touches them
                nc.vector.memset(
                    vha[kb][:, 0:HPC * 65].rearrange(
                        "p (h o) -> p h o", o=65)[:, :, 64:65], 1.0)
            attT = [res.tile([128, T], BF, tag=f"attT{p}", name=f"attT{p}")
                    for p in range(NPAIR)]

            if stages.startswith('attn'):
                for p in range(NPAIR):
                    nc.vector.memset(qhT[p][:], 0.01)
                    nc.vector.memset(khT[p][:], 0.01)
                    nc.vector.memset(attT[p][:], 0.0)
                for kb in range(NTKB):
                    nc.vector.memset(vha[kb][:], 0.01)

            # ---- projections (tchunk-major frees xT tiles early) -----------
            def qk_proj(key, w_tiles, dstT):
                for j in range(NTQ):
                    for p in range(NPAIR):
                        ps = mmA.tile([128, 512], F32, tag="mmA", name="mmA")
                        for cb in range(NCB):
                            nc.tensor.matmul(
                                ps[:],
                                lhsT=w_tiles[cb][:, p * 128:(p + 1) * 128],
                                rhs=xT[(key, j, cb)][:],
                                start=(cb == 0), stop=(cb == NCB - 1))
                        nc.vector.tensor_copy(
                            dstT[p][:, j * 512:(j + 1) * 512], ps[:])

            if not stages.startswith('attn'):
                qk_proj("q", wq_sb, qhT)
            wkall = res.tile([128, NCB * HD], BF, tag="wkall", name="wkall")
            load_w(wkall, wk, NCB, HD, nc.scalar)
            wk_sb = [wkall[:, cb * HD:(cb + 1) * HD] for cb in range(NCB)]
            if not stages.startswith('attn'):
                for j in range(NTQ):
                    load_xT("k", xk, j)
                qk_proj("k", wk_sb, khT)

            wvall = res.tile([128, NCB * HD], BF, tag="wvall", name="wvall")
            load_w(wvall, wv, NCB, HD, nc.scalar)
            wv_sb = [wvall[:, cb * HD:(cb + 1) * HD] for cb in range(NCB)]
            woall = res.tile([128, (HD // 128) * C], BF, tag="woall",
                             name="woall")
            load_w(woall, wo, HD // 128, C, nc.scalar)
            wo_sb = [woall[:, hb * C:(hb + 1) * C] for hb in range(HD // 128)]
            # v-proj emitted as small thunks: chunks 0-1 upfront, 2-3 as
            # fillers inside the attention loop (keeps PE busy while exp
            # paces the inner loop, and spreads the v transposes out).
            def vproj_thunks(kb, pool=None, tag="mm"):
                st = {}

                def mk_mm(cb):
                    def f():
                        if cb == 0:
                            st["ps"] = pool.tile([128, 512], F32, tag=tag,
                                                 name=tag)
                        nc.tensor.matmul(
                            st["ps"][:],
                            lhsT=xT[("v", kb // 4, cb)][:, (kb % 4) * 128:
                                                        (kb % 4 + 1) * 128],
                            rhs=wv_sb[cb][:],
                            start=(cb == 0), stop=(cb == NCB - 1))
                    return f

                def evac():
                    nc.vector.tensor_copy(
                        vha[kb][:, 0:HPC * 65].rearrange(
                            "p (h o) -> p h o", o=65)[:, :, 0:64],
                        st["ps"][:].rearrange("p (h o) -> p h o", o=64))

                return [mk_mm(cb) for cb in range(NCB)] + [evac]

            n_vchunk_upfront = NTQ if stages == 'proj' else 2
            if not stages.startswith('attn'):
                for j in range(n_vchunk_upfront):
                    load_xT("v", xv, j)
                for kb in range(4 * n_vchunk_upfront):
                    for f in vproj_thunks(kb, pool=mmA, tag="mmA"):
                        f()

            # ---- phase switch: release proj PSUM, open attention pools -----
            mmA.release()
            lgp = ctx.enter_context(tc.tile_pool(name="lg", bufs=2,
                                                 space="PSUM"))
            accp = ctx.enter_context(tc.tile_pool(name="acc", bufs=3,
                                                  space="PSUM"))
            mmp = ctx.enter_context(tc.tile_pool(name="mm", bufs=1,
                                                 space="PSUM"))

            # ---- attention + interleaved fillers ---------------------------
            filler = []

            def pop_fillers(n=2):
                for _ in range(min(n, len(filler))):
                    filler.pop(0)()

            def attention(p, j):
                c1 = (2 * p) * 65
                c2 = (2 * p + 1) * 65
                ntk = 4 * (j + 1)
                acc1 = accp.tile([65, 512], F32, tag="acc", name="acc")
                acc2 = accp.tile([65, 512], F32, tag="acc", name="acc")
                pend = []

                def emit_pv(it):
                    kb, o, w, pt = it
                    nc.tensor.matmul(
                        acc1[:, o:512], lhsT=vha[kb][:, c1:c1 + 65],
                        rhs=pt[:, 0:w],
                        start=(kb == 0), stop=(kb == ntk - 1))
                    nc.tensor.matmul(
                        acc2[:, o:512], lhsT=vha[kb][:, c2:c2 + 65],
                        rhs=pt[:, 512:512 + w],
                        start=(kb == 0), stop=(kb == ntk - 1))

                for kb in range(ntk):
                    o = max(0, kb * 128 - j * 512)
                    w = 512 - o
                    lg = lgp.tile([128, 1024], F32, tag="lg", name="lg")
                    nc.tensor.matmul(
                        lg[:, 0:w],
                        lhsT=khT[p][0:64, kb * 128:(kb + 1) * 128],
                        rhs=qhT[p][0:64, j * 512 + o:(j + 1) * 512],
                        start=True, stop=True)
                    nc.tensor.matmul(
                        lg[:, 512:512 + w],
                        lhsT=khT[p][64:128, kb * 128:(kb + 1) * 128],
                        rhs=qhT[p][64:128, j * 512 + o:(j + 1) * 512],
                        start=True, stop=True)
                    pt = ptp.tile([128, 1024], BF, tag="pt", name="pt")
                    if kb >= 4 * j:  # diagonal block: 2 narrow exps + 0/1 mask
                        nc.scalar.activation(
                            pt[:, 0:w], lg[:, 0:w], EXP, scale=SCALE)
                        nc.scalar.activation(
                            pt[:, 512:512 + w], lg[:, 512:512 + w], EXP,
                            scale=SCALE)
                        nc.vector.tensor_mul(
                            pt[:, 0:128], pt[:, 0:128], maskT[:])
                        nc.vector.tensor_mul(
                            pt[:, 512:640], pt[:, 512:640], maskT[:])
                    else:            # one exp covering both heads
                        nc.scalar.activation(
                            pt[:, 0:1024], lg[:, 0:1024], EXP, scale=SCALE)
                    pend.append((kb, o, w, pt))
                    pop_fillers(2)
                    if len(pend) > PIPE:
                        emit_pv(pend.pop(0))
                for it in pend:
                    emit_pv(it)

                # normalize: att^T = pv^T * (1/S), bf16
                if stages == 'attn_nonorm':
                    for acc in (acc1, acc2):
                        inv = sbp.tile([1, 512], F32, tag="inv", name="inv")
                        nc.vector.tensor_copy(inv[:], acc[64:65, :])
                elif stages == 'attn_nm2':
                    for acc, row in ((acc1, 0), (acc2, 64)):
                        pv = sbp.tile([65, 512], F32, tag="pvE", name="pvE")
                        nc.vector.tensor_copy(pv[:], acc[:])
                        lns = sbp.tile([1, 512], F32, tag="lns", name="lns")
                        nc.scalar.activation(
                            lns[:], pv[64:65, :],
                            mybir.ActivationFunctionType.Ln)
                        inv = sbp.tile([1, 512], F32, tag="inv", name="inv")
                        nc.scalar.activation(
                            inv[:], lns[:],
                            mybir.ActivationFunctionType.Exp, scale=-1.0)
                else:
                    # 1/S as exp(-ln S) on ACT: DVE's iterative-divide
                    # reciprocal is ~8 cyc/elem on one lane and blocks
                    # the DVE FIFO for ~4us.  Both heads' pv land in one
                    # [128,512] staging tile and both 1/S rows in one
                    # [2,512] tile so the broadcast is a single K=2
                    # matmul and the scale a single [128,512] mul.
                    pvS = sbp.tile([128, 512], F32, tag="pvE", name="pvE")
                    invs = []
                    for acc, base in ((acc1, 0), (acc2, 64)):
                        nc.vector.tensor_copy(
                            pvS[base:base + 64, :], acc[0:64, :])
                        lns = sbp.tile([1, 512], F32, tag="lns", name="lns")
                        nc.scalar.activation(
                            lns[:], acc[64:65, :],
                            mybir.ActivationFunctionType.Ln)
                        inv = sbp.tile([1, 512], BF, tag="inv", name="inv")
                        nc.scalar.activation(
                            inv[:], lns[:],
                            mybir.ActivationFunctionType.Exp, scale=-1.0)
                        invs.append(inv)

                    # defer broadcast+scale: a bcast matmul emitted here
                    # would stall PE on the ACT ln/exp chain.  Both heads
                    # broadcast into one [128,512] PSUM tile -> one mul.
                    def norm_thunk(pvS=pvS, invs=invs, p=p, j=j):
                        rep = mmp.tile([128, 512], F32, tag="mm",
                                       name="mm")
                        nc.tensor.matmul(rep[0:64, :], lhsT=ones64[:],
                                         rhs=invs[0][:],
                                         start=True, stop=True)
                        nc.tensor.matmul(rep[64:128, :], lhsT=ones64[:],
                                         rhs=invs[1][:],
                                         start=True, stop=True)
                        nc.vector.tensor_mul(
                            attT[p][:, j * 512:(j + 1) * 512],
                            pvS[:], rep[:])
                    filler.insert(0, norm_thunk)

            def outproj_thunks(tb):
                st = {}
                NHB = HD // 128

                def mk_mm(cc, hb):
                    def f():
                        if cc == 0 and hb == 0:
                            st["st"] = outp.tile([128, C], F32, tag="outS",
                                                 name="outS")
                        if hb == 0:
                            st["ps"] = mmp.tile([128, 512], F32, tag="mm",
                                                name="mm")
                        nc.tensor.matmul(
                            st["ps"][:],
                            lhsT=attT[hb][:, tb * 128:(tb + 1) * 128],
                            rhs=wo_sb[hb][:, cc * 512:(cc + 1) * 512],
                            start=(hb == 0), stop=(hb == NHB - 1))
                    return f

                def mk_evac(cc):
                    def f():
                        nc.vector.tensor_copy(
                            st["st"][:, cc * 512:(cc + 1) * 512], st["ps"][:])
                        if cc == C // 512 - 1:
                            nc.scalar.dma_start(
                                out[tb * 128:(tb + 1) * 128, :], st["st"][:])
                    return f

                th = []
                for cc in range(C // 512):
                    th.extend(mk_mm(cc, hb) for hb in range(NHB))
                    th.append(mk_evac(cc))
                return th

            if stages == 'proj':
                for p in range(NPAIR):
                    nc.vector.memset(attT[p][:], 0.0)
                for tb in range(T // 128):
                    for f in outproj_thunks(tb):
                        f()
            else:
                nofill = (stages == 'attn_nofill')
                for j in range(NTQ):
                    if not stages.startswith('attn') and j + 2 < NTQ:
                        load_xT("v", xv, j + 2)
                    if not stages.startswith('attn'):
                        for kb in range(4 * (j + 2), 4 * (j + 3)):
                            if kb < NTKB:
                                filler.extend(vproj_thunks(kb, pool=mmp, tag="mm"))
                    if j > 0 and not nofill:
                        for tb in range(4 * (j - 1), 4 * j):
                            filler.extend(outproj_thunks(tb))
                    for p in range(NPAIR):
                        attention(p, j)
                if nofill:
                    for tb in range(NTKB):
                        filler.extend(outproj_thunks(tb))
                else:
                    for tb in range(4 * (NTQ - 1), NTKB):
                        filler.extend(outproj_thunks(tb))
                while filler:
                    pop_fillers(8)

    nc.compile()
    _CACHE[key] = nc
    return nc


def make_in_maps(q, k, v, Wq, Wk, Wv, Wo):
    q = np.asarray(q, np.float32)
    k = np.asarray(k, np.float32)
    v = np.asarray(v, np.float32)
    Wq = np.asarray(Wq, np.float32)
    Wk = np.asarray(Wk, np.float32)
    Wv = np.asarray(Wv, np.float32)
    Wo = np.asarray(Wo, np.float32)

    def wslice(W, g):  # [H,C,D] -> [C, 8*D] for head group g
        return np.ascontiguousarray(
            W[g * HPC:(g + 1) * HPC].transpose(1, 0, 2).reshape(C, HD)
        ).astype(BF16)

    maps = []
    for core in range(NCORES):
        b, g = core // 2, core % 2
        maps.append({
            "xq": q[b].astype(BF16),
            "xk": k[b].astype(BF16),
            "xv": v[b].astype(BF16),
            "wq": wslice(Wq, g),
            "wk": wslice(Wk, g),
            "wv": wslice(Wv, g),
            "wo": np.ascontiguousarray(Wo[g * HD:(g + 1) * HD]).astype(BF16),
        })
    return maps


def kernel(q, k, v, Wq, Wk, Wv, Wo, bo):
    from concourse.bass_utils import run_bass_kernel_spmd

    nc = build_program()
    in_maps = make_in_maps(q, k, v, Wq, Wk, Wv, Wo)
    res = run_bass_kernel_spmd(nc, in_maps, list(range(NCORES))).results
    bo = np.asarray(bo, np.float32)
    outv = np.empty((B, T, C), np.float32)
    for b in range(B):
        outv[b] = res[2 * b]["out"] + res[2 * b + 1]["out"]
    outv += bo
    return outv



# revision 33
# speedup vs baseline: 3.8341x; 1.3397x over previous
"""Bass/Tile TRN2 kernel for nn_MultiHead (B=4, T=2048, C=1024, H=16, D=64).

Sharding: 8 cores = batch(4) x head-group(2).  Each core computes, for its
batch b and its 8 heads, the full attention block and a *partial* output
projection (its 512 rows of Wo).  Host sums the two partials per batch and
adds the bias.

On-device layout trick: all activations are kept transposed
([feature, time]) so every matmul sees natural-layout operands:
  - q/k/v are DMA-transposed on load (bf16, xbar transpose), in 512-col chunks
  - q-proj / k-proj:  lhsT = Wq[cb]  (c,hd),  rhs = qT[cb] (c,t)  -> qh^T (hd,t)
  - v-proj:           lhsT = vT[cb]  (c,tk),  rhs = Wv[cb] (c,hd) -> vh (tk,hd)
  - QK^T:             lhsT = kh^T    (d,tk),  rhs = qh^T   (d,tq) -> logits^T (tk,tq)
    (two heads run concurrently via PE row-tiling: d=64 halves of the array;
     both land in one 2-bank PSUM tile so one ACT exp covers both heads)
  - softmax along partitions: ones-column appended to vh gives row sums for
    free inside the PV matmul; causal mask added before exp; exp on ACT
  - PV:               lhsT = vh_aug  (tk,65), rhs = P^T    (tk,tq) -> [pv^T; S] (65,tq)
  - normalize: inv = 1/S broadcast to 64 partitions via a K=1 matmul
  - out-proj:         lhsT = att^T   (hd,tq), rhs = Wo     (hd,c)  -> out (tq,c)
Attention runs tq-chunk-major with the out-projection interleaved between
head-pairs to fill PE bubbles in the ACT(exp)-bound inner loop.
"""

import numpy as np
import ml_dtypes

B, T, C, H, D = 4, 2048, 1024, 16, 64
NCORES = 8
HPC = H // 2            # heads per core
HD = HPC * D            # 512, hidden per core
NPAIR = HPC // 2        # 4 head pairs
NCB = C // 128          # 8 contraction blocks
NTQ = T // 512          # 4 tq chunks
NTKB = T // 128         # 16 tk blocks
SCALE = float(1.0 / np.sqrt(np.float32(C)))
PIPE = 3                # QK->PV software pipeline depth
POPN = 4                # fillers popped per attention kb iteration

BF16 = ml_dtypes.bfloat16

_CACHE = {}


def build_program(repeat=1, stages='all'):
    """Build + compile the per-core Bass program (cached)."""
    key = ("nc", repeat, stages)
    if key in _CACHE:
        return _CACHE[key]

    import concourse.mybir as mybir
    import concourse.tile as tile
    from concourse import bacc
    from contextlib import ExitStack

    # Exp and Ln both live in the natural_log_exp_and_others table set, but
    # the table-load pass maps each func to the first set containing it
    # (Exp->exp_and_others, Ln->natural_log), reloading tables (~1.3us each)
    # at every Exp<->Ln switch.  Narrow the func->set map (dict order and
    # hence act_func_set_id indices unchanged) so both resolve to the shared
    # set and the load hoists out of the loop.
    from concourse.hw_specs import get_activation_tables
    _tabs = get_activation_tables("gen3")
    for _name, _funcs in _tabs.items():
        if _name != "natural_log_exp_and_others":
            _funcs.discard(mybir.ActivationFunctionType.Exp)
            _funcs.discard(mybir.ActivationFunctionType.Ln)

    BF = mybir.dt.bfloat16
    F32 = mybir.dt.float32

    nc = bacc.Bacc("TRN2", target_bir_lowering=False, debug=False,
                   enable_asserts=False, num_devices=NCORES)

    # q/k/v arrive host-pre-transposed ([C,T]) and weights pre-packed into
    # the SBUF-resident layout, so every load is a plain contiguous DMA
    # (no xbar transpose, no strided descriptors).
    xq = nc.dram_tensor("xq", [C, T], BF, kind="ExternalInput").ap()
    xk = nc.dram_tensor("xk", [C, T], BF, kind="ExternalInput").ap()
    xv = nc.dram_tensor("xv", [C, T], BF, kind="ExternalInput").ap()
    wq = nc.dram_tensor("wq", [128, NCB * HD], BF, kind="ExternalInput").ap()
    wk = nc.dram_tensor("wk", [128, NCB * HD], BF, kind="ExternalInput").ap()
    wv = nc.dram_tensor("wv", [128, NCB * HD], BF, kind="ExternalInput").ap()
    wo = nc.dram_tensor("wo", [128, (HD // 128) * C], BF,
                        kind="ExternalInput").ap()
    out = nc.dram_tensor("out", [T, C], BF, kind="ExternalOutput").ap()

    EXP = mybir.ActivationFunctionType.Exp

    with tile.TileContext(nc) as tc:
        with ExitStack() as ctx:
            if repeat > 1:
                ctx.enter_context(tc.For_i(0, repeat, 1))
            res = ctx.enter_context(tc.tile_pool(name="res", bufs=1))
            xTp = ctx.enter_context(tc.tile_pool(name="xT", bufs=48))
            # 48 = two full chunk generations of q/k/v tiles: chunk
            # j+1 loads must not alias chunk-j tiles, whose reader
            # thunks pop after the loads are emitted
            ptp = ctx.enter_context(tc.tile_pool(name="pt", bufs=4))
            sbp = ctx.enter_context(tc.tile_pool(name="sb", bufs=4))
            outp = ctx.enter_context(tc.tile_pool(name="outS", bufs=2))
            mmA = tc.alloc_tile_pool(name="mmA", bufs=2, space="PSUM")

            # ---- transposed input loads (kick q's first chunk ASAP) --------
            xT = {}   # (tensor, tchunk, cb) -> tile

            def load_xT(key, src, j, eng=None):
                for cb in range(NCB):
                    t = xTp.tile([128, 512], BF, tag="xT", name="xT")
                    (eng or nc.sync).dma_start(
                        t[:],
                        src[cb * 128:(cb + 1) * 128,
                            j * 512:(j + 1) * 512])
                    xT[(key, j, cb)] = t

            # ---- weights (wq first; others interleaved with loads) ----
            # pre-packed on host: one flat contiguous DMA per weight tensor
            def load_w(dst, src_ap, nblk, width, eng):
                eng.dma_start(dst[:], src_ap)

            wqall = res.tile([128, NCB * HD], BF, tag="wqall", name="wqall")
            load_w(wqall, wq, NCB, HD, nc.gpsimd)
            wq_sb = [wqall[:, cb * HD:(cb + 1) * HD] for cb in range(NCB)]
            if not stages.startswith('attn'):
                for j in range(NTQ if stages == 'proj' else 1):
                    load_xT("q", xq, j)

            # ---- constants --------------------------------------------------
            maskT = res.tile([128, 128], BF, tag="maskT", name="maskT")
            nc.gpsimd.memset(maskT[:], 1.0)
            # maskT[i,j] = 0 where j < i (strictly-lower = future in [tk,tq])
            nc.gpsimd.affine_select(
                out=maskT[:], in_=maskT[:],
                compare_op=mybir.AluOpType.is_ge, fill=0.0,
                base=0, pattern=[[1, 128]], channel_multiplier=-1,
            )
            ones64 = res.tile([1, 64], BF, tag="ones64", name="ones64")
            nc.vector.memset(ones64[:], 1.0)

            # ---- resident activations --------------------------------------
            qhT = [res.tile([128, T], BF, tag=f"qhT{p}", name=f"qhT{p}")
                   for p in range(NPAIR)]
            khT = [res.tile([128, T], BF, tag=f"khT{p}", name=f"khT{p}")
                   for p in range(NPAIR)]
            vha = [res.tile([128, HPC * 65], BF, tag=f"vha{kb}", name=f"vha{kb}")
                   for kb in range(NTKB)]
            for kb in range(NTKB):
                # ones columns (col 64 of each head's 65-wide strip) are
                # written once here; the v-proj evac never touches them
                nc.vector.memset(
                    vha[kb][:, 0:HPC * 65].rearrange(
                        "p (h o) -> p h o", o=65)[:, :, 64:65], 1.0)
            attT = [res.tile([128, T], BF, tag=f"attT{p}", name=f"attT{p}")
                    for p in range(NPAIR)]

            if stages.startswith('attn'):
                for p in range(NPAIR):
                    nc.vector.memset(qhT[p][:], 0.01)
                    nc.vector.memset(khT[p][:], 0.01)
                    nc.vector.memset(attT[p][:], 0.0)
                for kb in range(NTKB):
                    nc.vector.memset(vha[kb][:], 0.01)

            # ---- projections ------------------------------------------------
            # only pair 0 of q/k proj chunk 0 runs upfront (inline, mmA
            # pool); everything else (pairs 1-3, v-proj, chunks 1-3) is
            # queued as fillers inside the attention loop, so the ACT-bound
            # attention pipeline starts as soon as qh/kh pair 0 exist.
            def qkproj_pair_thunks(key, w_tiles, dstT, j, p, pool, tag):
                st = {}

                def mk_mm(cb):
                    def f():
                        if cb == 0:
                            st["ps"] = pool.tile([128, 512], F32,
                                                 tag=tag, name=tag)
                        nc.tensor.matmul(
                            st["ps"][:],
                            lhsT=w_tiles[cb][:, p * 128:(p + 1) * 128],
                            rhs=xT[(key, j, cb)][:],
                            start=(cb == 0), stop=(cb == NCB - 1))
                    return f

                def evac():
                    nc.vector.tensor_copy(
                        dstT[p][:, j * 512:(j + 1) * 512], st["ps"][:])

                return [mk_mm(cb) for cb in range(NCB)] + [evac]

            def qkproj_thunks(key, w_tiles, dstT, j, pool, tag):
                th = []
                for p in range(NPAIR):
                    th.extend(qkproj_pair_thunks(key, w_tiles, dstT, j, p,
                                                 pool, tag))
                return th

            wkall = res.tile([128, NCB * HD], BF, tag="wkall", name="wkall")
            load_w(wkall, wk, NCB, HD, nc.gpsimd)
            wk_sb = [wkall[:, cb * HD:(cb + 1) * HD] for cb in range(NCB)]
            n_upfront = NTQ if stages == 'proj' else 1
            if not stages.startswith('attn'):
                # chunk-0 k transposes go on the ACT hwdge queue (idle until
                # the first exp) so they don't serialize behind q (SP
                # queue); v follows q on SP.
                load_xT("k", xk, 0, eng=nc.scalar)
                for j in range(n_upfront):
                    if j > 0:
                        load_xT("k", xk, j)
                    for p in range(NPAIR if stages == 'proj' else 1):
                        for f in qkproj_pair_thunks("q", wq_sb, qhT, j, p,
                                                    mmA, "mmA"):
                            f()
                        for f in qkproj_pair_thunks("k", wk_sb, khT, j, p,
                                                    mmA, "mmA"):
                            f()

            wvall = res.tile([128, NCB * HD], BF, tag="wvall", name="wvall")
            load_w(wvall, wv, NCB, HD, nc.gpsimd)
            wv_sb = [wvall[:, cb * HD:(cb + 1) * HD] for cb in range(NCB)]
            woall = res.tile([128, (HD // 128) * C], BF, tag="woall",
                             name="woall")
            load_w(woall, wo, HD // 128, C, nc.gpsimd)
            wo_sb = [woall[:, hb * C:(hb + 1) * C] for hb in range(HD // 128)]
            # v-proj emitted as small thunks: chunks 0-1 upfront, 2-3 as
            # fillers inside the attention loop (keeps PE busy while exp
            # paces the inner loop, and spreads the v transposes out).
            def vproj_thunks(kb, pool=None, tag="mm"):
                st = {}

                def mk_mm(cb):
                    def f():
                        if cb == 0:
                            st["ps"] = pool.tile([128, 512], F32, tag=tag,
                                                 name=tag)
                        nc.tensor.matmul(
                            st["ps"][:],
                            lhsT=xT[("v", kb // 4, cb)][:, (kb % 4) * 128:
                                                        (kb % 4 + 1) * 128],
                            rhs=wv_sb[cb][:],
                            start=(cb == 0), stop=(cb == NCB - 1))
                    return f

                def evac():
                    nc.vector.tensor_copy(
                        vha[kb][:, 0:HPC * 65].rearrange(
                            "p (h o) -> p h o", o=65)[:, :, 0:64],
                        st["ps"][:].rearrange("p (h o) -> p h o", o=64))

                return [mk_mm(cb) for cb in range(NCB)] + [evac]

            if stages == 'proj':
                for j in range(NTQ):
                    load_xT("v", xv, j)
                for kb in range(4 * NTQ):
                    for f in vproj_thunks(kb, pool=mmA, tag="mmA"):
                        f()
            elif not stages.startswith('attn'):
                load_xT("v", xv, 0)

            # ---- phase switch: release proj PSUM, open attention pools -----
            mmA.release()
            lgp = ctx.enter_context(tc.tile_pool(name="lg", bufs=2,
                                                 space="PSUM"))
            accp = ctx.enter_context(tc.tile_pool(name="acc", bufs=3,
                                                  space="PSUM"))
            mmp = ctx.enter_context(tc.tile_pool(name="mm", bufs=1,
                                                 space="PSUM"))

            # ---- attention + interleaved fillers ---------------------------
            # Filler thunks EMIT instructions when popped, and the tile
            # dependency tracker orders by emission: a consumer emitted
            # before its producer thunk pops would read stale data.  Pushes
            # are therefore named groups, and ensure(name) force-pops the
            # queue (FIFO) through that group before the consumer is
            # emitted.
            filler = []
            fstate = {"pushed": 0, "popped": 0}
            done_at = {}

            def push_group(name, thunks):
                filler.extend(thunks)
                fstate["pushed"] += len(thunks)
                if name:
                    done_at[name] = fstate["pushed"]

            def pop_fillers(n=2):
                for _ in range(min(n, len(filler))):
                    filler.pop(0)()
                    fstate["popped"] += 1

            def ensure(name):
                while fstate["popped"] < done_at.get(name, 0):
                    pop_fillers(1)

            if stages not in ('proj',) and not stages.startswith('attn'):
                # rest of chunk 0: pairs 1-3 of q/k proj and v-proj kb 0-3,
                # interleaved so each pair's inputs land just ahead of its
                # attention() call
                for kb in range(4):
                    push_group(f"v{kb}", vproj_thunks(kb, pool=mmp,
                                                      tag="mm"))
                for p in range(1, NPAIR):
                    push_group(f"q{p}c0",
                               qkproj_pair_thunks("q", wq_sb, qhT, 0, p,
                                                  mmp, "mm"))
                    push_group(f"k{p}c0",
                               qkproj_pair_thunks("k", wk_sb, khT, 0, p,
                                                  mmp, "mm"))

            def attention(p, j):
                c1 = (2 * p) * 65
                c2 = (2 * p + 1) * 65
                ntk = 4 * (j + 1)
                ensure(f"q{p}c{j}")
                ensure(f"k{p}c{j}")
                acc1 = accp.tile([65, 512], F32, tag="acc", name="acc")
                acc2 = accp.tile([65, 512], F32, tag="acc", name="acc")
                pend = []

                def emit_pv(it):
                    kb, o, w, pt = it
                    ensure(f"v{kb}")
                    nc.tensor.matmul(
                        acc1[:, o:512], lhsT=vha[kb][:, c1:c1 + 65],
                        rhs=pt[:, 0:w],
                        start=(kb == 0), stop=(kb == ntk - 1))
                    nc.tensor.matmul(
                        acc2[:, o:512], lhsT=vha[kb][:, c2:c2 + 65],
                        rhs=pt[:, 512:512 + w],
                        start=(kb == 0), stop=(kb == ntk - 1))

                for kb in range(ntk):
                    o = max(0, kb * 128 - j * 512)
                    w = 512 - o
                    lg = lgp.tile([128, 1024], F32, tag="lg", name="lg")
                    nc.tensor.matmul(
                        lg[:, 0:w],
                        lhsT=khT[p][0:64, kb * 128:(kb + 1) * 128],
                        rhs=qhT[p][0:64, j * 512 + o:(j + 1) * 512],
                        start=True, stop=True)
                    nc.tensor.matmul(
                        lg[:, 512:512 + w],
                        lhsT=khT[p][64:128, kb * 128:(kb + 1) * 128],
                        rhs=qhT[p][64:128, j * 512 + o:(j + 1) * 512],
                        start=True, stop=True)
                    pt = ptp.tile([128, 1024], BF, tag="pt", name="pt")
                    if kb >= 4 * j:  # diagonal block: 2 narrow exps + 0/1 mask
                        nc.scalar.activation(
                            pt[:, 0:w], lg[:, 0:w], EXP, scale=SCALE)
                        nc.scalar.activation(
                            pt[:, 512:512 + w], lg[:, 512:512 + w], EXP,
                            scale=SCALE)
                        nc.vector.tensor_mul(
                            pt[:, 0:128], pt[:, 0:128], maskT[:])
                        nc.vector.tensor_mul(
                            pt[:, 512:640], pt[:, 512:640], maskT[:])
                    else:            # one exp covering both heads
                        nc.scalar.activation(
                            pt[:, 0:1024], lg[:, 0:1024], EXP, scale=SCALE)
                    pend.append((kb, o, w, pt))
                    pop_fillers(POPN)
                    if len(pend) > PIPE:
                        emit_pv(pend.pop(0))
                for it in pend:
                    emit_pv(it)

                # normalize: att^T = pv^T * (1/S), bf16
                if stages == 'attn_nonorm':
                    for acc in (acc1, acc2):
                        inv = sbp.tile([1, 512], F32, tag="inv", name="inv")
                        nc.vector.tensor_copy(inv[:], acc[64:65, :])
                elif stages == 'attn_nm2':
                    for acc, row in ((acc1, 0), (acc2, 64)):
                        pv = sbp.tile([65, 512], F32, tag="pvE", name="pvE")
                        nc.vector.tensor_copy(pv[:], acc[:])
                        lns = sbp.tile([1, 512], F32, tag="lns", name="lns")
                        nc.scalar.activation(
                            lns[:], pv[64:65, :],
                            mybir.ActivationFunctionType.Ln)
                        inv = sbp.tile([1, 512], F32, tag="inv", name="inv")
                        nc.scalar.activation(
                            inv[:], lns[:],
                            mybir.ActivationFunctionType.Exp, scale=-1.0)
                else:
                    # 1/S as exp(-ln S) on ACT: DVE's iterative-divide
                    # reciprocal is ~8 cyc/elem on one lane and blocks
                    # the DVE FIFO for ~4us.  Both heads' pv land in one
                    # [128,512] staging tile and both 1/S rows in one
                    # [2,512] tile so the broadcast is a single K=2
                    # matmul and the scale a single [128,512] mul.
                    pvS = sbp.tile([128, 512], BF, tag="pvE", name="pvE")
                    invs = []
                    for acc, base in ((acc1, 0), (acc2, 64)):
                        nc.vector.tensor_copy(
                            pvS[base:base + 64, :], acc[0:64, :])
                        lns = sbp.tile([1, 512], F32, tag="lns", name="lns")
                        nc.scalar.activation(
                            lns[:], acc[64:65, :],
                            mybir.ActivationFunctionType.Ln)
                        inv = sbp.tile([1, 512], BF, tag="inv", name="inv")
                        nc.scalar.activation(
                            inv[:], lns[:],
                            mybir.ActivationFunctionType.Exp, scale=-1.0)
                        invs.append(inv)

                    # defer broadcast+scale: a bcast matmul emitted here
                    # would stall PE on the ACT ln/exp chain.  Both heads
                    # broadcast into one [128,512] PSUM tile -> one mul.
                    def norm_thunk(pvS=pvS, invs=invs, p=p, j=j):
                        rep = mmp.tile([128, 512], F32, tag="mm",
                                       name="mm")
                        nc.tensor.matmul(rep[0:64, :], lhsT=ones64[:],
                                         rhs=invs[0][:],
                                         start=True, stop=True)
                        nc.tensor.matmul(rep[64:128, :], lhsT=ones64[:],
                                         rhs=invs[1][:],
                                         start=True, stop=True)
                        nc.vector.tensor_mul(
                            attT[p][:, j * 512:(j + 1) * 512],
                            pvS[:], rep[:])
                    push_group(f"n{p}c{j}", [norm_thunk])

            def outproj_thunks(tb):
                st = {}
                NHB = HD // 128

                def mk_mm(cc, hb):
                    def f():
                        if cc == 0 and hb == 0:
                            st["st"] = outp.tile([128, C], BF, tag="outS",
                                                 name="outS")
                        if hb == 0:
                            st["ps"] = mmp.tile([128, 512], F32, tag="mm",
                                                name="mm")
                        nc.tensor.matmul(
                            st["ps"][:],
                            lhsT=attT[hb][:, tb * 128:(tb + 1) * 128],
                            rhs=wo_sb[hb][:, cc * 512:(cc + 1) * 512],
                            start=(hb == 0), stop=(hb == NHB - 1))
                    return f

                def mk_evac(cc):
                    def f():
                        nc.vector.tensor_copy(
                            st["st"][:, cc * 512:(cc + 1) * 512], st["ps"][:])
                        if cc == C // 512 - 1:
                            nc.gpsimd.dma_start(
                                out[tb * 128:(tb + 1) * 128, :], st["st"][:])
                    return f

                th = []
                for cc in range(C // 512):
                    th.extend(mk_mm(cc, hb) for hb in range(NHB))
                    th.append(mk_evac(cc))
                return th

            if stages == 'proj':
                for p in range(NPAIR):
                    nc.vector.memset(attT[p][:], 0.0)
                for tb in range(T // 128):
                    for f in outproj_thunks(tb):
                        f()
            else:
                nofill = (stages == 'attn_nofill')
                for j in range(NTQ):
                    if not stages.startswith('attn') and j + 1 < NTQ:
                        load_xT("q", xq, j + 1)
                        load_xT("k", xk, j + 1)
                        load_xT("v", xv, j + 1)
                        for p in range(NPAIR):
                            push_group(
                                f"q{p}c{j + 1}",
                                qkproj_pair_thunks("q", wq_sb, qhT, j + 1,
                                                   p, mmp, "mm"))
                            push_group(
                                f"k{p}c{j + 1}",
                                qkproj_pair_thunks("k", wk_sb, khT, j + 1,
                                                   p, mmp, "mm"))
                            push_group(
                                f"v{4 * (j + 1) + p}",
                                vproj_thunks(4 * (j + 1) + p, pool=mmp,
                                             tag="mm"))
                    if j > 0 and not nofill:
                        for tb in range(4 * (j - 1), 4 * j):
                            push_group(None, outproj_thunks(tb))
                    for p in range(NPAIR):
                        attention(p, j)
                if nofill:
                    for tb in range(NTKB):
                        push_group(None, outproj_thunks(tb))
                else:
                    for tb in range(4 * (NTQ - 1), NTKB):
                        push_group(None, outproj_thunks(tb))
                while filler:
                    pop_fillers(8)

    nc.compile()
    _CACHE[key] = nc
    return nc


def make_in_maps(q, k, v, Wq, Wk, Wv, Wo):
    q = np.asarray(q, np.float32)
    k = np.asarray(k, np.float32)
    v = np.asarray(v, np.float32)
    Wq = np.asarray(Wq, np.float32)
    Wk = np.asarray(Wk, np.float32)
    Wv = np.asarray(Wv, np.float32)
    Wo = np.asarray(Wo, np.float32)

    def wslice(W, g):
        # [H,C,D] -> [C, 8*D] for head group g -> packed [128, NCB*HD]
        w = W[g * HPC:(g + 1) * HPC].transpose(1, 0, 2).reshape(C, HD)
        return np.ascontiguousarray(
            w.reshape(NCB, 128, HD).transpose(1, 0, 2).reshape(
                128, NCB * HD)).astype(BF16)

    def woslice(g):
        # [HD, C] rows for group g -> packed [128, (HD//128)*C]
        w = Wo[g * HD:(g + 1) * HD]
        return np.ascontiguousarray(
            w.reshape(HD // 128, 128, C).transpose(1, 0, 2).reshape(
                128, (HD // 128) * C)).astype(BF16)

    maps = []
    xqs = [np.ascontiguousarray(q[b].T).astype(BF16) for b in range(B)]
    xks = [np.ascontiguousarray(k[b].T).astype(BF16) for b in range(B)]
    xvs = [np.ascontiguousarray(v[b].T).astype(BF16) for b in range(B)]
    ws = [(wslice(Wq, g), wslice(Wk, g), wslice(Wv, g), woslice(g))
          for g in range(2)]
    for core in range(NCORES):
        b, g = core // 2, core % 2
        maps.append({
            "xq": xqs[b],
            "xk": xks[b],
            "xv": xvs[b],
            "wq": ws[g][0],
            "wk": ws[g][1],
            "wv": ws[g][2],
            "wo": ws[g][3],
        })
    return maps


def kernel(q, k, v, Wq, Wk, Wv, Wo, bo):
    from concourse.bass_utils import run_bass_kernel_spmd

    nc = build_program()
    in_maps = make_in_maps(q, k, v, Wq, Wk, Wv, Wo)
    res = run_bass_kernel_spmd(nc, in_maps, list(range(NCORES))).results
    bo = np.asarray(bo, np.float32)
    outv = np.empty((B, T, C), np.float32)
    for b in range(B):
        outv[b] = (res[2 * b]["out"].astype(np.float32)
                   + res[2 * b + 1]["out"].astype(np.float32))
    outv += bo
    return outv

